# revision 10
# baseline (speedup 1.0000x reference)
"""Differentiable-stack kernel for Trainium2 (Bass/Tile), 8-core data parallel.

The reference soft stack only ever reads slot S-1, and the shift moves slot
s+1 -> slot s (never upward), so the output reduces to a gated linear
recurrence per (batch, d):

    y_t = a_t * y_{t-1} + b_t * x_t
    a_t = (1-p_t)(1-o_t),  b_t = p_t (1-o_t)      (scalars per (b, t))

Per core: 2 batch elements of [L=2048, D=512] f32.  The recurrence is
computed chunk-wise on the TensorEngine as a banded triangular matmul with
an exact rank-1 cross-chunk carry:

    y[s:s+128] = C_c^T . x[s:s+128]  +  P_c (x) y[s-1]
    C_c[j,t]   = b_j * prod_{k=j+1..t} a_k   (j<=t, within chunk)
               = exp(S_t - S_j + ln b_j)     (S = in-chunk cumsum of ln a)
    P_c[t]     = prod_{k=s..t} a_k = exp(S_t)

C_c is built per chunk with one rank-1 PSUM broadcast of the S row, a
constant "ramp" matmul (-1000*max(j-t,0)) that suppresses the j>t half
before the exp, and a single ScalarE EXP activation with a per-partition
bias column (ln b_j - S_j) obtained from one PE transpose of the gate
tensor.  x is cast f32->bf16 on the fly by SWDGE DMA; outputs are computed
in bf16 and cast back to f32 by SWDGE store DMA (HBM traffic is the fixed
cost; bf16 keeps every on-chip pass at half cost and PE at full rate).
"""

import os
from contextlib import ExitStack

import numpy as np

import concourse.bass as bass
import concourse.tile as tile
from concourse import bacc, mybir
from concourse.bass_utils import run_bass_kernel_spmd

F32 = mybir.dt.float32
BF16 = mybir.dt.bfloat16
ALU = mybir.AluOpType
ACTF = mybir.ActivationFunctionType

B, L, D = 16, 2048, 512
NCORES = 8
BPC = B // NCORES            # batches per core = 2
T = 128                      # chunk length (rows per matmul)
NC = L // T                  # chunks per batch element = 16
SEG = BPC * NC               # gate-tensor partitions = 32

# DMA grouping (chunks per load/store call), front-loaded small for fast start
LGROUPS = [int(g) for g in os.environ.get("DSTACK_LG", "2,2,4,4,4").split(",")]
SGROUPS = [int(g) for g in os.environ.get("DSTACK_SG", "2,2,4,4,4").split(",")]
YBUFS = int(os.environ.get("DSTACK_YBUFS", "3"))
PSYBUFS = int(os.environ.get("DSTACK_PSY", "4"))
CTBUFS = int(os.environ.get("DSTACK_CT", "4"))
# fraction of the psum->sbuf y copy done on DVE (rest on ScalarE)
DVE_COLS = int(os.environ.get("DSTACK_DVECOLS", "192"))

assert sum(LGROUPS) == NC and sum(SGROUPS) == NC


def build_module():
    nc = bacc.Bacc("TRN2", target_bir_lowering=False)
    xin = nc.dram_tensor("xin", [T, BPC * NC * D], F32, kind="ExternalInput")
    pg = nc.dram_tensor("pg", [SEG, T], F32, kind="ExternalInput")
    og = nc.dram_tensor("og", [SEG, T], F32, kind="ExternalInput")
    yout = nc.dram_tensor("yout", [T, BPC * NC * D], F32, kind="ExternalOutput")
    # DRAM bounce buffers: re-layout [SEG, T] gate vectors as partition-0 rows
    # (matmul operands must sit at base partition 0/32/64)
    scr_s = nc.dram_tensor("scr_s", [1, SEG * T], F32, kind="Internal")
    scr_p = nc.dram_tensor("scr_p", [1, SEG * T], BF16, kind="Internal")

    with tile.TileContext(nc) as tc, ExitStack() as ctx:
        smalls = ctx.enter_context(tc.tile_pool(name="smalls", bufs=1))
        xpool = ctx.enter_context(tc.tile_pool(name="xpool", bufs=1))
        ypool = ctx.enter_context(tc.tile_pool(name="ypool", bufs=YBUFS))
        ctpool = ctx.enter_context(tc.tile_pool(name="ctpool", bufs=CTBUFS))
        carrypool = ctx.enter_context(tc.tile_pool(name="carrypool", bufs=2))
        pspool = ctx.enter_context(tc.tile_pool(name="pspool", bufs=2, space="PSUM"))

        # -------- gate DMAs (HWDGE sync queue: fast, independent ring) ------
        pgt = smalls.tile([SEG, T], F32)
        ogt = smalls.tile([SEG, T], F32)
        nc.sync.dma_start(pgt[:], pg[:])
        nc.sync.dma_start(ogt[:], og[:])

        # -------- x cast-loads (SWDGE, f32 HBM -> bf16 SBUF) ---------------
        # All emitted before any other gpsimd op so Q7 emits every load
        # descriptor up front; SDMA drains while compute proceeds.
        xtiles = {}          # (b, c) -> (tile, col0)
        for b in range(BPC):
            c0 = 0
            for gi, g in enumerate(LGROUPS):
                xt = xpool.tile([T, g * D], BF16, tag=f"x{b}_{gi}")
                for c in range(c0, c0 + g):
                    xtiles[(b, c)] = (xt, (c - c0) * D)
                c0 += g
        for gi, g in enumerate(LGROUPS):
            c0 = sum(LGROUPS[:gi])
            for b in range(BPC):
                xt = xtiles[(b, c0)][0]
                lo = (b * NC + c0) * D
                nc.gpsimd.dma_start(xt[:], xin[:, lo:lo + g * D])

        # -------- constants (gpsimd Q7, after load descriptor emission) ----
        ident32 = smalls.tile([32, 32], F32)
        nc.gpsimd.memset(ident32[:], 0.0)
        nc.gpsimd.affine_select(
            out=ident32[:], in_=ident32[:], compare_op=ALU.not_equal, fill=1.0,
            base=0, pattern=[[-1, 32]], channel_multiplier=1)
        # Output rows are ROTATED: psum row p holds timestep t'=p-1 (row 0 ->
        # t'=127) so the cross-chunk carry lands at partition 0.  Ramp
        # weights: lhsT L[k,j]=1{k<j}, rhs U[k,p]=-1000*1{k >= t'(p)}:
        # psum += -1000*max(j - t'(p), 0), killing the j>t' half before exp.
        # Column p=0 is t'=127 where every j is valid -> zero column.
        lmat = smalls.tile([T, T], BF16)
        nc.gpsimd.memset(lmat[:], 1.0)
        nc.gpsimd.affine_select(
            out=lmat[:], in_=lmat[:], compare_op=ALU.is_ge, fill=0.0,
            base=-1, pattern=[[1, T]], channel_multiplier=-1)
        umat = smalls.tile([T, T], BF16)
        nc.gpsimd.memset(umat[:], -1000.0)
        nc.gpsimd.affine_select(
            out=umat[:], in_=umat[:], compare_op=ALU.is_ge, fill=0.0,
            base=1, pattern=[[-1, T]], channel_multiplier=1)
        nc.gpsimd.affine_select(
            out=umat[:], in_=umat[:], compare_op=ALU.is_ge, fill=0.0,
            base=-1, pattern=[[1, T]], channel_multiplier=0)

        # -------- gate math (tiny, [SEG, T]) -------------------------------
        ones_st = smalls.tile([SEG, T], F32)
        nc.vector.memset(ones_st[:], 1.0)
        ones_row = smalls.tile([1, T], F32)
        nc.vector.memset(ones_row[:], 1.0)

        om = smalls.tile([SEG, T], F32)
        av = smalls.tile([SEG, T], F32)
        bv = smalls.tile([SEG, T], F32)
        nc.vector.tensor_scalar(om[:], ogt[:], -1.0, 1.0, ALU.mult, ALU.add)
        nc.vector.tensor_scalar(av[:], pgt[:], -1.0, 1.0, ALU.mult, ALU.add)
        nc.vector.tensor_mul(av[:], av[:], om[:])
        nc.vector.tensor_mul(bv[:], pgt[:], om[:])

        la = smalls.tile([SEG, T], F32)
        nc.scalar.activation(la[:], av[:], ACTF.Ln)
        lb = smalls.tile([SEG, T], F32)
        nc.scalar.activation(lb[:], bv[:], ACTF.Ln)

        # S = within-segment inclusive cumsum of ln a
        sv = smalls.tile([SEG, T], F32)
        nc.vector.tensor_tensor_scan(sv[:], ones_st[:], la[:], 0.0,
                                     ALU.mult, ALU.add)
        # bias column source: ln b - S
        bias = smalls.tile([SEG, T], F32)
        nc.vector.tensor_sub(bias[:], lb[:], sv[:])
        # P row: exp(S) in bf16 (rank-1 carry weights)
        prow = smalls.tile([SEG, T], BF16)
        nc.scalar.activation(prow[:], sv[:], ACTF.Exp)

        # bounce S and P through DRAM to get partition-0 row layouts, stored
        # in rotated column order (col 0 = t'=127, col p = t'=p-1)
        srows = smalls.tile([1, SEG * T], F32)
        prows = smalls.tile([1, SEG * T], BF16)
        scr_s2 = scr_s[:].rearrange("o (p f) -> (o p) f", f=T)
        scr_p2 = scr_p[:].rearrange("o (p f) -> (o p) f", f=T)
        nc.sync.dma_start(scr_s2[:, 0:1], sv[:, T - 1:T])
        nc.sync.dma_start(scr_s2[:, 1:T], sv[:, 0:T - 1])
        nc.sync.dma_start(scr_p2[:, 0:1], prow[:, T - 1:T])
        nc.sync.dma_start(scr_p2[:, 1:T], prow[:, 0:T - 1])
        nc.sync.dma_start(srows[:], scr_s[:])
        nc.sync.dma_start(prows[:], scr_p[:])

        # biasT[t', seg] via one PE transpose
        bias_ps = pspool.tile([T, SEG], F32, tag="btp", bufs=1)
        nc.tensor.transpose(bias_ps[:], bias[:], ident32[:])
        biast = smalls.tile([T, SEG], F32)
        nc.scalar.copy(biast[:], bias_ps[:])

        # -------- main loop ------------------------------------------------
        carries = {}
        store_plan = []      # (b, gi, c0, g) emitted when last chunk done
        sgrp = {}
        for b in range(BPC):
            c0 = 0
            for gi, g in enumerate(SGROUPS):
                yt = ypool.tile([T, g * D], BF16, tag=f"y{b}")
                for c in range(c0, c0 + g):
                    sgrp[(b, c)] = (yt, (c - c0) * D, c == c0 + g - 1,
                                    (b * NC + c0) * D, g)
                c0 += g

        for c in range(NC):
            for b in range(BPC):
                seg = b * NC + c
                # C^T tile: psum2 = bcast(S row) + ramp; Ct = exp(psum2+bias)
                psum2 = pspool.tile([T, T], F32, tag="p2", bufs=2)
                nc.tensor.matmul(psum2[:], ones_row[:],
                                 srows[0:1, seg * T:(seg + 1) * T],
                                 start=True, stop=False)
                nc.tensor.matmul(psum2[:], lmat[:], umat[:],
                                 start=False, stop=True, skip_group_check=True)
                ct = ctpool.tile([T, T], BF16, tag="ct")
                nc.scalar.activation(ct[:], psum2[:], ACTF.Exp,
                                     bias=biast[:, seg:seg + 1], scale=1.0)

                xt, xcol = xtiles[(b, c)]
                psy = pspool.tile([T, D], F32, tag="psy", bufs=PSYBUFS)
                first = (c == 0)
                nc.tensor.matmul(psy[:], ct[:], xt[:, xcol:xcol + D],
                                 start=True, stop=first)
                if not first:
                    nc.tensor.matmul(psy[:], prows[0:1, seg * T:(seg + 1) * T],
                                     carries[b][:], start=False, stop=True,
                                     skip_group_check=True)

                # next carry first (short ACT op, unblocks next chunk's PE);
                # rotated rows put the last timestep at partition 0
                if c < NC - 1:
                    cw = carrypool.tile([1, D], BF16, tag=f"cw{b}")
                    nc.scalar.copy(cw[:], psy[0:1, :])
                    carries[b] = cw

                # psum -> sbuf (split ScalarE / DVE)
                yt, ycol, last, dcol0, g = sgrp[(b, c)]
                nsc = D - DVE_COLS
                nc.scalar.copy(yt[:, ycol:ycol + nsc], psy[:, 0:nsc])
                if DVE_COLS:
                    nc.vector.tensor_copy(yt[:, ycol + nsc:ycol + D],
                                          psy[:, nsc:D])
                if last:
                    # un-rotate at store time: row p holds t'=p-1, row 0 holds
                    # t'=127
                    nc.gpsimd.dma_start(yout[0:T - 1, dcol0:dcol0 + g * D],
                                        yt[1:T, :])
                    nc.gpsimd.dma_start(yout[T - 1:T, dcol0:dcol0 + g * D],
                                        yt[0:1, :])

    nc.compile()
    return nc


_module_cache = {}


def _get_module():
    if "nc" not in _module_cache:
        _module_cache["nc"] = build_module()
    return _module_cache["nc"]


def make_in_maps(x, push_gate, pop_gate):
    x = np.ascontiguousarray(np.asarray(x), dtype=np.float32)
    pgf = np.asarray(push_gate, dtype=np.float32).reshape(B, L)
    ogf = np.asarray(pop_gate, dtype=np.float32).reshape(B, L)
    in_maps = []
    for i in range(NCORES):
        sl = slice(i * BPC, (i + 1) * BPC)
        xi = x[sl].reshape(BPC, NC, T, D).transpose(2, 0, 1, 3)
        in_maps.append({
            "xin": np.ascontiguousarray(xi.reshape(T, BPC * NC * D)),
            "pg": np.ascontiguousarray(pgf[sl].reshape(SEG, T)),
            "og": np.ascontiguousarray(ogf[sl].reshape(SEG, T)),
        })
    return in_maps


def run(x, push_gate, pop_gate, **spmd_kwargs):
    """Run on hardware; returns (output, BassKernelResults)."""
    nc = _get_module()
    in_maps = make_in_maps(x, push_gate, pop_gate)
    res = run_bass_kernel_spmd(nc, in_maps, core_ids=list(range(NCORES)),
                               **spmd_kwargs)
    outs = []
    for i in range(NCORES):
        yo = res.results[i]["yout"].reshape(T, BPC, NC, D)
        outs.append(yo.transpose(1, 2, 0, 3).reshape(BPC, L, D))
    return np.concatenate(outs, axis=0), res


def kernel(x, push_gate, pop_gate):
    out, _ = run(x, push_gate, pop_gate)
    return out
